# revision 50
# baseline (speedup 1.0000x reference)
"""Causal self-attention (B=4, T=2048, C=1024, NH=16) on 8 TRN2 NeuronCores.

Sharding: core c -> batch b = c//2, head-group g = c%2 (8 heads, Dh=512).
Each core computes q/k/v projections for its head group on its batch,
fused causal attention, and a partial output projection through its
row-slice of Wp.  Host sums the two partials per batch.

Design (all bf16 matmul inputs, fp32 psum; 312us vs the 457us f32r
baseline; fp8 QK was tried and rejected at 3e-2 > 2e-2 tolerance):
  - Softmax exp merged per k-chunk into one [128, 1024] activation
    covering both head parities, reading a persistent double-buffered
    psum pair (qkA/qkB) so QK(kc+2) overlaps exp(kc).
  - Causal masking: tri-strip multiply on the 128-wide diagonal
    boundary only, after exp; sub-diagonal regions are simply never
    read by the (row-trimmed) AV and QK matmuls.
  - Softmax denominators ride the AV matmul via ones-columns in the v
    slots, are gathered through a staging tile + tiny SBUF-SBUF DMA
    into per-q-block [8, 512] tiles, inverted with one DVE reciprocal,
    and broadcast back to 128 rows via selection-mask matmuls.
  - Emission is software-pipelined: projection of block ib+1 and
    normalization of block jq-1 are round-robin interleaved between
    attention chunk groups; all out-projections are deferred into the
    last (ACT-heaviest) section as PE filler.
  - Per-chunk x/weight tiles + interleaved DMA order let the first
    matmul start ~1.5us in; output is written bf16 and the two
    head-group partials per batch are summed on the host in f32.

kernel(**inputs) takes the FULL unsharded inputs and returns the FULL
output.  Self-contained: hardcodes all shapes, reads nothing from disk.
"""

import sys

sys.path.insert(0, "/opt/trn_rl_repo")

import numpy as np
import ml_dtypes
from contextlib import ExitStack

import concourse.bass as bass  # noqa: F401
import concourse.mybir as mybir
import concourse.tile as tile
from concourse import bacc
from concourse.bass_utils import run_bass_kernel_spmd

P = 128
B, T, C = 4, 2048, 1024
NH, HS = 16, 64
D = 512          # per-core head dim (8 heads)
H = 8            # local heads
f32 = mybir.dt.float32
bf16 = mybir.dt.bfloat16
f8 = mybir.dt.float8e4
DR = mybir.MatmulPerfMode.DoubleRow
AFT = mybir.ActivationFunctionType


def build_nc(t=T):
    assert t % 512 == 0
    nq = t // 512     # q blocks of 512
    nkc = t // 128    # k chunks of 128
    nco = C // P      # contraction chunks (8)

    nc = bacc.Bacc("TRN2", target_bir_lowering=False, debug=False, num_devices=8)

    xt_d = nc.dram_tensor("xt", [C, t], bf16, kind="ExternalInput")
    wq_d = nc.dram_tensor("wq", [C, D], bf16, kind="ExternalInput")
    wk_d = nc.dram_tensor("wk", [C, D], bf16, kind="ExternalInput")
    wv_d = nc.dram_tensor("wv", [C, D], bf16, kind="ExternalInput")
    wp_d = nc.dram_tensor("wp", [D, C], bf16, kind="ExternalInput")
    tri_d = nc.dram_tensor("tri", [P, P], bf16, kind="ExternalInput")
    selm_d = nc.dram_tensor("selm", [8, 4, P], bf16, kind="ExternalInput")
    out_d = nc.dram_tensor("out", [t, C], bf16, kind="ExternalOutput")

    xt_r = xt_d[:].rearrange("(co p) t -> p co t", p=P)
    wq_r = wq_d[:].rearrange("(co p) d -> p co d", p=P)
    wk_r = wk_d[:].rearrange("(co p) d -> p co d", p=P)
    wv_r = wv_d[:].rearrange("(co p) d -> p co d", p=P)
    wp_r = wp_d[:].rearrange("(dc p) c -> p dc c", p=P)
    out_r = out_d[:].rearrange("(tc p) c -> p tc c", p=P)

    with tile.TileContext(nc) as tc, ExitStack() as ctx, nc.allow_low_precision(
        reason="bf16 attention kernel"
    ):
        sb = ctx.enter_context(tc.tile_pool(name="sb", bufs=1))
        psum = ctx.enter_context(tc.tile_pool(name="psum", bufs=1, space="PSUM"))

        qt_sb = sb.tile([P, 4, t], bf16)
        kt_sb = sb.tile([P, 4, t], bf16)
        v_sb = sb.tile([P, nkc, H, P], bf16)
        yt_sb = sb.tile([P, 4, t], bf16)
        # per-jq softmax-denominator tiles: row r = hp*2 + par, par0=odd head
        sg_sb = [
            sb.tile([8, 512], f32, tag=f"sg{j}", name=f"sg{j}") for j in range(nq)
        ]
        sgi_sb = [
            sb.tile([8, 512], bf16, tag=f"sgi{j}", name=f"sgi{j}")
            for j in range(nq)
        ]
        tri_sb = sb.tile([P, P], bf16)
        selm_sb = sb.tile([8, 4, P], bf16)
        # per-chunk weight/x tiles so the first matmul waits on 2 DMAs,
        # not 16 (dependencies are tracked per tile)
        wq_sb = [sb.tile([P, D], bf16, name=f"wq{c}") for c in range(nco)]
        wk_sb = [sb.tile([P, D], bf16, name=f"wk{c}") for c in range(nco)]
        wv_sb = sb.tile([P, nco, D], bf16)
        wp_sb = sb.tile([P, 4, C], bf16)

        v5 = v_sb[:].rearrange("p k (hp par) c -> p k hp par c", par=2)

        # ---------- prelude: DMAs + memsets ----------
        # interleave x-block-0 chunks with the q/k weights they pair with
        # so the first projection matmul starts after ~2 small DMAs
        xtb0 = [
            sb.tile([P, 512], bf16, tag=f"xtb{c}", bufs=2, name=f"xtb0_{c}")
            for c in range(nco)
        ]
        for co in range(nco):
            nc.sync.dma_start(xtb0[co][:], xt_r[:, co, 0:512])
            nc.sync.dma_start(wq_sb[co][:], wq_r[:, co, :])
            nc.sync.dma_start(wk_sb[co][:], wk_r[:, co, :])
        for co in range(nco):
            nc.sync.dma_start(wv_sb[:, co, :], wv_r[:, co, :])
        nc.sync.dma_start(tri_sb[:], tri_d[:])
        nc.sync.dma_start(selm_sb[:], selm_d[:])
        nc.sync.dma_start(wp_sb[:], wp_r)
        # persistent double-buffered QK psum pair: alternating chunks use
        # A/B; stale sub-diagonal regions are re-read by exp (finite old
        # logits) and masked out after, so diag QK matmuls can trim
        qkA = psum.tile([P, 1024], f32, tag="qka", bufs=1)
        qkB = psum.tile([P, 1024], f32, tag="qkb", bufs=1)
        nc.gpsimd.memset(v_sb[:], 0.0)
        nc.gpsimd.memset(v5[:, :, :, 0, 64:65], 1.0)  # even head: s at row 64
        nc.gpsimd.memset(v5[:, :, :, 1, 0:1], 1.0)    # odd head: s at row 0
        for j in range(nq):
            nc.vector.memset(sg_sb[j][:], 1.0)
            nc.vector.memset(sgi_sb[j][:], 1.0)

        # ---------- emission units ----------
        def proj_units(ib):
            """Projection of x block ib -> qt/kt/v.  13 units."""
            if ib == 0:
                xtb = xtb0
            else:
                xtb = [
                    sb.tile(
                        [P, 512], bf16, tag=f"xtb{c}", bufs=2, name=f"xtb{ib}_{c}"
                    )
                    for c in range(nco)
                ]
                for co in range(nco):
                    nc.sync.dma_start(
                        xtb[co][:], xt_r[:, co, ib * 512 : (ib + 1) * 512]
                    )
                yield
            for m in range(4):
                for w_sb, dst in ((wq_sb, qt_sb), (wk_sb, kt_sb)):
                    ps = psum.tile([P, 512], f32, tag="io", bufs=2)
                    for co in range(nco):
                        nc.tensor.matmul(
                            ps[:],
                            w_sb[co][:, m * P : (m + 1) * P],
                            xtb[co][:],
                            start=(co == 0),
                            stop=(co == nco - 1),
                        )
                    nc.vector.tensor_copy(
                        out=dst[:, m, ib * 512 : (ib + 1) * 512], in_=ps[:]
                    )
                    yield
            for t4 in range(4):
                kc = ib * 4 + t4
                ps = psum.tile([P, 512], f32, tag="io", bufs=2)
                for co in range(nco):
                    nc.tensor.matmul(
                        ps[:],
                        xtb[co][:, t4 * P : (t4 + 1) * P],
                        wv_sb[:, co, :],
                        start=(co == 0),
                        stop=(co == nco - 1),
                    )
                src = ps[:].rearrange("p (hp par c) -> p hp par c", par=2, c=64)
                nc.vector.tensor_copy(out=v5[:, kc, :, 0, 0:64], in_=src[:, :, 0, :])
                nc.vector.tensor_copy(
                    out=v5[:, kc, :, 1, 64:128], in_=src[:, :, 1, :]
                )
                yield

        def att_units(hp, jq):
            """Attention for head-pair hp on q block jq.

            One k-chunk per group: QK-E + QK-O into a double-buffered
            [P, 1024] psum, one exp over both parities.  Diagonal chunks
            get small memset + tri-strip masking (off the QK->exp chain).
            AV trails the exp by one chunk.
            """
            q0 = jq * 512
            nk = 4 * (jq + 1)
            psyE = psum.tile([P, 512], f32, tag="ye", bufs=1)
            psyO = psum.tile([P, 512], f32, tag="yo", bufs=1)

            def av_emit(kc, attg):
                d = kc - 4 * jq
                off = 128 * d if d >= 0 else 0
                for par, psy in ((0, psyE), (1, psyO)):
                    nc.tensor.matmul(
                        psy[:, off:512],
                        v_sb[:, kc, 2 * hp + par, :],
                        attg[:, par * 512 + off : par * 512 + 512],
                        start=(kc == 0),
                        stop=(kc == nk - 1),
                        skip_group_check=True,
                    )

            prev = None
            for kc in range(nk):
                d = kc - 4 * jq
                # trim diag QK on jq>0 (jq=0 writes full so the psum pair's
                # first generations are fully initialized)
                toff = 128 * d if (d > 0 and jq > 0) else 0
                qk = qkA if kc % 2 == 0 else qkB
                for par, sl in ((0, slice(0, 64)), (1, slice(64, 128))):
                    col = par * 512
                    nc.tensor.matmul(
                        qk[:, col + toff : col + 512],
                        kt_sb[sl, hp, kc * P : (kc + 1) * P],
                        qt_sb[sl, hp, q0 + toff : q0 + 512],
                        start=True,
                        stop=True,
                    )
                attg = sb.tile([P, 1024], bf16, tag="att", bufs=4)
                nc.scalar.activation(attg[:], qk[:], AFT.Exp, scale=0.125)
                if d >= 0:  # diagonal chunk -> causal tri mask on the
                    # boundary strip; cols below off are never read by AV
                    off = 128 * d
                    for par in (0, 1):
                        col = par * 512
                        eng = nc.vector if par == 0 else nc.gpsimd
                        eng.tensor_mul(
                            out=attg[:, col + off : col + off + P],
                            in0=attg[:, col + off : col + off + P],
                            in1=tri_sb[:],
                        )
                if prev is not None:
                    av_emit(*prev)
                prev = (kc, attg)
                yield
            av_emit(*prev)
            # drains: yt + s rows (staged, then a tiny DMA gathers psum
            # rows {0 (odd s), 64 (even s)} into adjacent sg rows)
            nc.vector.tensor_copy(
                out=yt_sb[0:64, hp, q0 : q0 + 512], in_=psyE[0:64, :]
            )
            nc.vector.tensor_copy(
                out=yt_sb[64:128, hp, q0 : q0 + 512], in_=psyO[64:128, :]
            )
            stmp = sb.tile([P, 512], f32, tag="stmp", bufs=2)
            nc.vector.tensor_copy(out=stmp[64:65, :], in_=psyE[64:65, :])
            nc.vector.tensor_copy(out=stmp[0:1, :], in_=psyO[0:1, :])
            nc.sync.dma_start(
                sg_sb[jq][hp * 2 : hp * 2 + 2, :], stmp[0:65:64, :]
            )
            yield

        def tail_units(jq):
            yield from norm_units(jq)
            yield from outp_units(jq)

        def norm_units(jq):
            """Normalization (1/s broadcast + scale) for q block jq."""
            q0 = jq * 512
            # 1/s on DVE (keeps the ACT table set pinned to exp)
            nc.vector.reciprocal(out=sgi_sb[jq][:], in_=sg_sb[jq][:])
            yield
            for hp in range(4):
                rb = psum.tile([P, 512], f32, tag="io", bufs=2)
                nc.tensor.matmul(
                    rb[:],
                    selm_sb[:, hp, :],
                    sgi_sb[jq][:],
                    start=True,
                    stop=True,
                )
                nc.vector.tensor_mul(
                    out=yt_sb[:, hp, q0 : q0 + 512],
                    in0=yt_sb[:, hp, q0 : q0 + 512],
                    in1=rb[:],
                )
                yield

        def outp_units(jq):
            """Out-projection for q block jq (deferrable filler work)."""
            for tcn in range(jq * 4, jq * 4 + 4):
                ob = sb.tile([P, C], bf16, tag="ob", bufs=3)
                for n2 in (0, 1):
                    pso = psum.tile([P, 512], f32, tag="io", bufs=2)
                    for dc in range(4):
                        nc.tensor.matmul(
                            pso[:],
                            yt_sb[:, dc, tcn * P : (tcn + 1) * P],
                            wp_sb[:, dc, n2 * 512 : (n2 + 1) * 512],
                            start=(dc == 0),
                            stop=(dc == 3),
                        )
                    if n2 == 0:
                        nc.vector.tensor_copy(out=ob[:, 0:512], in_=pso[:])
                    else:
                        nc.scalar.copy(out=ob[:, 512:1024], in_=pso[:])
                eng = nc.sync if tcn % 2 == 0 else nc.scalar
                eng.dma_start(out_r[:, tcn, :], ob[:])
                yield

        def chain(*gens):
            for g in gens:
                yield from g

        def roundrobin(*gens):
            gens = list(gens)
            while gens:
                for g in list(gens):
                    try:
                        next(g)
                        yield
                    except StopIteration:
                        gens.remove(g)

        def run_merged(main_gens, filler_gens, n_main, n_fill, skip_main=0):
            main = chain(*main_gens)
            fill = chain(*filler_gens)
            ratio = (n_fill / max(1, n_main - skip_main)) if n_main else 0.0
            acc = 0.0
            done = object()
            fill_done = n_fill == 0
            k = 0
            for _ in main:
                k += 1
                if k <= skip_main:
                    continue
                acc += ratio
                while acc >= 1.0 and not fill_done:
                    fill_done = next(fill, done) is done
                    acc -= 1.0
            while not fill_done:
                fill_done = next(fill, done) is done

        # ---------- emission schedule ----------
        for _ in proj_units(0):
            pass
        for jq in range(nq):
            mains = [att_units(hp, jq) for hp in range(4)]
            n_main = 4 * (4 * (jq + 1) + 1)
            fillers = []
            n_fill = 0
            if jq + 1 < nq:
                fillers.append(proj_units(jq + 1))
                n_fill += 13
            if jq >= 1:
                fillers.append(norm_units(jq - 1))
                n_fill += 5
            if jq == nq - 1:
                # out-projections deferred here: the last section has the
                # most ACT (exp) work and the least other PE filler
                for j2 in range(nq - 1):
                    fillers.append(outp_units(j2))
                    n_fill += 4
            if len(fillers) > 1:
                fillers = [chain(*fillers)] if jq == nq - 1 else [
                    roundrobin(*fillers)
                ]
            # back-load fillers in the last section: the exp deficit
            # persists to its end, and early chunks there have AV filler
            run_merged(
                mains, fillers, n_main, n_fill,
                skip_main=(n_main // 3 if jq == nq - 1 else 0),
            )
        for _ in chain(norm_units(nq - 1), outp_units(nq - 1)):
            pass

    nc.finalize()
    return nc


_NC = None


def _get_nc():
    global _NC
    if _NC is None:
        _NC = build_nc()
    return _NC


def make_in_maps(x, Wk, Wq, Wv, Wp, t=T):
    x = np.asarray(x, dtype=np.float32)
    Wk = np.asarray(Wk, dtype=np.float32)
    Wq = np.asarray(Wq, dtype=np.float32)
    Wv = np.asarray(Wv, dtype=np.float32)
    Wp = np.asarray(Wp, dtype=np.float32)
    bf = ml_dtypes.bfloat16

    # lower-tri strip mask: tri[k, j] = 1 iff j >= k
    tri = np.triu(np.ones((P, P), np.float32)).astype(bf)

    # selection masks for broadcasting 1/s rows; sg row hp*2+0 holds the
    # ODD head's s (psum row 0), hp*2+1 the even head's (psum row 64)
    selm = np.zeros((8, 4, P), np.float32)
    for hp in range(4):
        selm[hp * 2 + 0, hp, 64:128] = 1.0
        selm[hp * 2 + 1, hp, 0:64] = 1.0
    selm = selm.astype(bf)

    in_maps = []
    for c in range(8):
        b, g = c // 2, c % 2
        sl = slice(g * D, (g + 1) * D)
        in_maps.append({
            "xt": np.ascontiguousarray(x[b, :t].T).astype(bf),
            "wq": np.ascontiguousarray(Wq[:, sl]).astype(bf),
            "wk": np.ascontiguousarray(Wk[:, sl]).astype(bf),
            "wv": np.ascontiguousarray(Wv[:, sl]).astype(bf),
            "wp": np.ascontiguousarray(Wp[sl, :]).astype(bf),
            "tri": tri,
            "selm": selm,
        })
    return in_maps


def _run(x, Wk, Wq, Wv, Wp, trace=False):
    nc = _get_nc()
    in_maps = make_in_maps(x, Wk, Wq, Wv, Wp)
    res = run_bass_kernel_spmd(nc, in_maps, core_ids=list(range(8)), trace=trace)
    parts = [
        np.asarray(res.results[c]["out"], dtype=np.float32) for c in range(8)
    ]
    out = np.stack(
        [parts[2 * b] + parts[2 * b + 1] for b in range(B)], axis=0
    )
    return out, res


def kernel(x, Wk, Wq, Wv, Wp):
    out, _ = _run(x, Wk, Wq, Wv, Wp, trace=False)
    return out


# revision 51
# speedup vs baseline: 1.1544x; 1.1544x over previous
"""Causal self-attention (B=4, T=2048, C=1024, NH=16) on 8 TRN2 NeuronCores.

Sharding: core c -> batch b = c//2, head-group g = c%2 (8 heads, Dh=512).
Each core computes q/k/v projections for its head group on its batch,
fused causal attention, and a partial output projection through its
row-slice of Wp.  Host sums the two partials per batch.

v2 layout (all bf16 matmul inputs, fp32 psum):
  - Softmax exp merged into [128, 2048] psum groups holding BOTH head
    parities x 2 k-chunks -> 4x fewer / 4x wider ACT instructions.
  - Causal masking via host-built tri masks multiplied AFTER exp
    (DVE/Pool), so the QK matmuls and exp run on full tiles with no
    per-chunk memset/off bookkeeping.
  - Softmax denominators gathered into one [32, 512] tile; ln/exp
    batched per q-block (8 rows at a time) instead of [1, 2048] strips.
  - 1/s broadcast back to 128 rows via tiny selection-mask matmuls.
  - Projection, attention, normalization and out-projection emission is
    software-pipelined: proj(ib+1)/outproj(jq-1) pieces are round-robin
    interleaved between attention groups so no engine sits idle between
    phases.
  - DMA order: first x block first, weights after, so the PE starts
    ~7us in instead of ~40us.

kernel(**inputs) takes the FULL unsharded inputs and returns the FULL
output.  Self-contained: hardcodes all shapes, reads nothing from disk.
"""

import sys

sys.path.insert(0, "/opt/trn_rl_repo")

import numpy as np
import ml_dtypes
from contextlib import ExitStack

import concourse.bass as bass  # noqa: F401
import concourse.mybir as mybir
import concourse.tile as tile
from concourse import bacc
from concourse.bass_utils import run_bass_kernel_spmd

P = 128
B, T, C = 4, 2048, 1024
NH, HS = 16, 64
D = 512          # per-core head dim (8 heads)
H = 8            # local heads
f32 = mybir.dt.float32
bf16 = mybir.dt.bfloat16
f8 = mybir.dt.float8e4
DR = mybir.MatmulPerfMode.DoubleRow
AFT = mybir.ActivationFunctionType


def build_nc(t=T):
    assert t % 512 == 0
    nq = t // 512     # q blocks of 512
    nkc = t // 128    # k chunks of 128
    nco = C // P      # contraction chunks (8)

    nc = bacc.Bacc("TRN2", target_bir_lowering=False, debug=False, num_devices=8)

    xt_d = nc.dram_tensor("xt", [C, t], bf16, kind="ExternalInput")
    wq_d = nc.dram_tensor("wq", [C, D], bf16, kind="ExternalInput")
    wk_d = nc.dram_tensor("wk", [C, D], bf16, kind="ExternalInput")
    wv_d = nc.dram_tensor("wv", [C, D], bf16, kind="ExternalInput")
    wp_d = nc.dram_tensor("wp", [D, C], bf16, kind="ExternalInput")
    tri_d = nc.dram_tensor("tri", [P, P], bf16, kind="ExternalInput")
    selm_d = nc.dram_tensor("selm", [8, 4, P], bf16, kind="ExternalInput")
    out_d = nc.dram_tensor("out", [t, C], bf16, kind="ExternalOutput")

    xt_r = xt_d[:].rearrange("(co p) t -> p co t", p=P)
    wq_r = wq_d[:].rearrange("(co p) d -> p co d", p=P)
    wk_r = wk_d[:].rearrange("(co p) d -> p co d", p=P)
    wv_r = wv_d[:].rearrange("(co p) d -> p co d", p=P)
    wp_r = wp_d[:].rearrange("(dc p) c -> p dc c", p=P)
    out_r = out_d[:].rearrange("(tc p) c -> p tc c", p=P)

    with tile.TileContext(nc) as tc, ExitStack() as ctx, nc.allow_low_precision(
        reason="bf16 attention kernel"
    ):
        sb = ctx.enter_context(tc.tile_pool(name="sb", bufs=1))
        psum = ctx.enter_context(tc.tile_pool(name="psum", bufs=1, space="PSUM"))

        qt_sb = sb.tile([P, 4, t], bf16)
        kt_sb = sb.tile([P, 4, t], bf16)
        v_sb = sb.tile([P, nkc, H, P], bf16)
        yt_sb = sb.tile([P, 4, t], bf16)
        # per-jq softmax-denominator tiles: row r = hp*2 + par, par0=odd head
        sg_sb = [
            sb.tile([8, 512], f32, tag=f"sg{j}", name=f"sg{j}") for j in range(nq)
        ]
        sgi_sb = [
            sb.tile([8, 512], bf16, tag=f"sgi{j}", name=f"sgi{j}")
            for j in range(nq)
        ]
        tri_sb = sb.tile([P, P], bf16)
        selm_sb = sb.tile([8, 4, P], bf16)
        # per-chunk weight/x tiles so the first matmul waits on 2 DMAs,
        # not 16 (dependencies are tracked per tile)
        wq_sb = [sb.tile([P, D], bf16, name=f"wq{c}") for c in range(nco)]
        wk_sb = [sb.tile([P, D], bf16, name=f"wk{c}") for c in range(nco)]
        wv_sb = sb.tile([P, nco, D], bf16)
        wp_sb = sb.tile([P, 4, C], bf16)

        v5 = v_sb[:].rearrange("p k (hp par) c -> p k hp par c", par=2)

        # ---------- prelude: DMAs + memsets ----------
        # interleave x-block-0 chunks with the q/k weights they pair with
        # so the first projection matmul starts after ~2 small DMAs
        xtb0 = [
            sb.tile([P, 512], bf16, tag=f"xtb{c}", bufs=2, name=f"xtb0_{c}")
            for c in range(nco)
        ]
        for co in range(nco):
            nc.sync.dma_start(xtb0[co][:], xt_r[:, co, 0:512])
            nc.sync.dma_start(wq_sb[co][:], wq_r[:, co, :])
            nc.sync.dma_start(wk_sb[co][:], wk_r[:, co, :])
        for co in range(nco):
            nc.sync.dma_start(wv_sb[:, co, :], wv_r[:, co, :])
        nc.sync.dma_start(tri_sb[:], tri_d[:])
        nc.sync.dma_start(selm_sb[:], selm_d[:])
        nc.sync.dma_start(wp_sb[:], wp_r)
        # persistent double-buffered QK psum pair: alternating chunks use
        # A/B; stale sub-diagonal regions are re-read by exp (finite old
        # logits) and masked out after, so diag QK matmuls can trim
        qkA = psum.tile([P, 1024], f32, tag="qka", bufs=1)
        qkB = psum.tile([P, 1024], f32, tag="qkb", bufs=1)
        nc.gpsimd.memset(v_sb[:], 0.0)
        nc.gpsimd.memset(v5[:, :, :, 0, 64:65], 1.0)  # even head: s at row 64
        nc.gpsimd.memset(v5[:, :, :, 1, 0:1], 1.0)    # odd head: s at row 0
        for j in range(nq):
            nc.vector.memset(sg_sb[j][:], 1.0)
            nc.vector.memset(sgi_sb[j][:], 1.0)

        # ---------- emission units ----------
        def proj_units(ib):
            """Projection of x block ib -> qt/kt/v.  13 units."""
            if ib == 0:
                xtb = xtb0
            else:
                xtb = [
                    sb.tile(
                        [P, 512], bf16, tag=f"xtb{c}", bufs=2, name=f"xtb{ib}_{c}"
                    )
                    for c in range(nco)
                ]
                for co in range(nco):
                    nc.sync.dma_start(
                        xtb[co][:], xt_r[:, co, ib * 512 : (ib + 1) * 512]
                    )
                yield
            for m in range(4):
                for w_sb, dst in ((wq_sb, qt_sb), (wk_sb, kt_sb)):
                    ps = psum.tile([P, 512], f32, tag="io", bufs=2)
                    for co in range(nco):
                        nc.tensor.matmul(
                            ps[:],
                            w_sb[co][:, m * P : (m + 1) * P],
                            xtb[co][:],
                            start=(co == 0),
                            stop=(co == nco - 1),
                        )
                    nc.vector.tensor_copy(
                        out=dst[:, m, ib * 512 : (ib + 1) * 512], in_=ps[:]
                    )
                    yield
            for t4 in range(4):
                kc = ib * 4 + t4
                ps = psum.tile([P, 512], f32, tag="io", bufs=2)
                for co in range(nco):
                    nc.tensor.matmul(
                        ps[:],
                        xtb[co][:, t4 * P : (t4 + 1) * P],
                        wv_sb[:, co, :],
                        start=(co == 0),
                        stop=(co == nco - 1),
                    )
                src = ps[:].rearrange("p (hp par c) -> p hp par c", par=2, c=64)
                nc.vector.tensor_copy(out=v5[:, kc, :, 0, 0:64], in_=src[:, :, 0, :])
                nc.vector.tensor_copy(
                    out=v5[:, kc, :, 1, 64:128], in_=src[:, :, 1, :]
                )
                yield

        def att_units(hp, jq):
            """Attention for head-pair hp on q block jq.

            One k-chunk per group: QK-E + QK-O into a double-buffered
            [P, 1024] psum, one exp over both parities.  Diagonal chunks
            get small memset + tri-strip masking (off the QK->exp chain).
            AV trails the exp by one chunk.
            """
            q0 = jq * 512
            nk = 4 * (jq + 1)
            psyE = psum.tile([P, 512], f32, tag="ye", bufs=1)
            psyO = psum.tile([P, 512], f32, tag="yo", bufs=1)

            def av_emit(kc, attg):
                d = kc - 4 * jq
                off = 128 * d if d >= 0 else 0
                for par, psy in ((0, psyE), (1, psyO)):
                    nc.tensor.matmul(
                        psy[:, off:512],
                        v_sb[:, kc, 2 * hp + par, :],
                        attg[:, par * 512 + off : par * 512 + 512],
                        start=(kc == 0),
                        stop=(kc == nk - 1),
                        skip_group_check=True,
                    )

            prev = None
            for kc in range(nk):
                d = kc - 4 * jq
                # trim diag QK on jq>0 (jq=0 writes full so the psum pair's
                # first generations are fully initialized)
                toff = 128 * d if (d > 0 and jq > 0) else 0
                qk = qkA if kc % 2 == 0 else qkB
                for par, sl in ((0, slice(0, 64)), (1, slice(64, 128))):
                    col = par * 512
                    nc.tensor.matmul(
                        qk[:, col + toff : col + 512],
                        kt_sb[sl, hp, kc * P : (kc + 1) * P],
                        qt_sb[sl, hp, q0 + toff : q0 + 512],
                        start=True,
                        stop=True,
                    )
                attg = sb.tile([P, 1024], bf16, tag="att", bufs=4)
                nc.scalar.activation(attg[:], qk[:], AFT.Exp, scale=0.125)
                if d >= 0:  # diagonal chunk -> causal tri mask on the
                    # boundary strip; cols below off are never read by AV
                    off = 128 * d
                    for par in (0, 1):
                        col = par * 512
                        eng = nc.vector if par == 0 else nc.gpsimd
                        eng.tensor_mul(
                            out=attg[:, col + off : col + off + P],
                            in0=attg[:, col + off : col + off + P],
                            in1=tri_sb[:],
                        )
                if prev is not None:
                    av_emit(*prev)
                prev = (kc, attg)
                yield
            av_emit(*prev)
            # drains: yt + s rows (staged, then a tiny DMA gathers psum
            # rows {0 (odd s), 64 (even s)} into adjacent sg rows)
            nc.vector.tensor_copy(
                out=yt_sb[0:64, hp, q0 : q0 + 512], in_=psyE[0:64, :]
            )
            nc.vector.tensor_copy(
                out=yt_sb[64:128, hp, q0 : q0 + 512], in_=psyO[64:128, :]
            )
            stmp = sb.tile([P, 512], f32, tag="stmp", bufs=2)
            nc.vector.tensor_copy(out=stmp[64:65, :], in_=psyE[64:65, :])
            nc.vector.tensor_copy(out=stmp[0:1, :], in_=psyO[0:1, :])
            nc.sync.dma_start(
                sg_sb[jq][hp * 2 : hp * 2 + 2, :], stmp[0:65:64, :]
            )
            yield

        def tail_units(jq):
            yield from norm_units(jq)
            yield from outp_units(jq)

        def norm_units(jq):
            """Normalization (1/s broadcast + scale) for q block jq."""
            q0 = jq * 512
            # 1/s on DVE (keeps the ACT table set pinned to exp)
            nc.vector.reciprocal(out=sgi_sb[jq][:], in_=sg_sb[jq][:])
            yield
            for hp in range(4):
                rb = psum.tile([P, 512], f32, tag="io", bufs=2)
                nc.tensor.matmul(
                    rb[:],
                    selm_sb[:, hp, :],
                    sgi_sb[jq][:],
                    start=True,
                    stop=True,
                )
                nc.vector.tensor_mul(
                    out=yt_sb[:, hp, q0 : q0 + 512],
                    in0=yt_sb[:, hp, q0 : q0 + 512],
                    in1=rb[:],
                )
                yield

        def outp_units(jq):
            """Out-projection for q block jq (deferrable filler work)."""
            for tcn in range(jq * 4, jq * 4 + 4):
                ob = sb.tile([P, C], bf16, tag="ob", bufs=3)
                for n2 in (0, 1):
                    pso = psum.tile([P, 512], f32, tag="io", bufs=2)
                    for dc in range(4):
                        nc.tensor.matmul(
                            pso[:],
                            yt_sb[:, dc, tcn * P : (tcn + 1) * P],
                            wp_sb[:, dc, n2 * 512 : (n2 + 1) * 512],
                            start=(dc == 0),
                            stop=(dc == 3),
                        )
                    if n2 == 0:
                        nc.vector.tensor_copy(out=ob[:, 0:512], in_=pso[:])
                    else:
                        nc.scalar.copy(out=ob[:, 512:1024], in_=pso[:])
                eng = nc.sync if tcn % 2 == 0 else nc.scalar
                eng.dma_start(out_r[:, tcn, :], ob[:])
                yield

        def chain(*gens):
            for g in gens:
                yield from g

        def roundrobin(*gens):
            gens = list(gens)
            while gens:
                for g in list(gens):
                    try:
                        next(g)
                        yield
                    except StopIteration:
                        gens.remove(g)

        def run_merged(main_gens, filler_gens, n_main, n_fill):
            main = chain(*main_gens)
            fill = chain(*filler_gens)
            ratio = (n_fill / n_main) if n_main else 0.0
            acc = 0.0
            done = object()
            fill_done = n_fill == 0
            for _ in main:
                acc += ratio
                while acc >= 1.0 and not fill_done:
                    fill_done = next(fill, done) is done
                    acc -= 1.0
            while not fill_done:
                fill_done = next(fill, done) is done

        # ---------- emission schedule ----------
        for _ in proj_units(0):
            pass
        for jq in range(nq):
            mains = [att_units(hp, jq) for hp in range(4)]
            n_main = 4 * (4 * (jq + 1) + 1)
            fillers = []
            n_fill = 0
            if jq + 1 < nq:
                fillers.append(proj_units(jq + 1))
                n_fill += 13
            if jq >= 1:
                fillers.append(norm_units(jq - 1))
                n_fill += 5
            if jq == nq - 1:
                # out-projections deferred here: the last section has the
                # most ACT (exp) work and the least other PE filler
                for j2 in range(nq - 1):
                    fillers.append(outp_units(j2))
                    n_fill += 4
            if len(fillers) > 1:
                fillers = [chain(*fillers)] if jq == nq - 1 else [
                    roundrobin(*fillers)
                ]
            run_merged(mains, fillers, n_main, n_fill)
        for _ in chain(norm_units(nq - 1), outp_units(nq - 1)):
            pass

    nc.finalize()
    return nc


_NC = None


def _get_nc():
    global _NC
    if _NC is None:
        _NC = build_nc()
    return _NC


def make_in_maps(x, Wk, Wq, Wv, Wp, t=T):
    x = np.asarray(x, dtype=np.float32)
    Wk = np.asarray(Wk, dtype=np.float32)
    Wq = np.asarray(Wq, dtype=np.float32)
    Wv = np.asarray(Wv, dtype=np.float32)
    Wp = np.asarray(Wp, dtype=np.float32)
    bf = ml_dtypes.bfloat16

    # lower-tri strip mask: tri[k, j] = 1 iff j >= k
    tri = np.triu(np.ones((P, P), np.float32)).astype(bf)

    # selection masks for broadcasting 1/s rows; sg row hp*2+0 holds the
    # ODD head's s (psum row 0), hp*2+1 the even head's (psum row 64)
    selm = np.zeros((8, 4, P), np.float32)
    for hp in range(4):
        selm[hp * 2 + 0, hp, 64:128] = 1.0
        selm[hp * 2 + 1, hp, 0:64] = 1.0
    selm = selm.astype(bf)

    in_maps = []
    for c in range(8):
        b, g = c // 2, c % 2
        sl = slice(g * D, (g + 1) * D)
        in_maps.append({
            "xt": np.ascontiguousarray(x[b, :t].T).astype(bf),
            "wq": np.ascontiguousarray(Wq[:, sl]).astype(bf),
            "wk": np.ascontiguousarray(Wk[:, sl]).astype(bf),
            "wv": np.ascontiguousarray(Wv[:, sl]).astype(bf),
            "wp": np.ascontiguousarray(Wp[sl, :]).astype(bf),
            "tri": tri,
            "selm": selm,
        })
    return in_maps


def _run(x, Wk, Wq, Wv, Wp, trace=False):
    nc = _get_nc()
    in_maps = make_in_maps(x, Wk, Wq, Wv, Wp)
    res = run_bass_kernel_spmd(nc, in_maps, core_ids=list(range(8)), trace=trace)
    parts = [
        np.asarray(res.results[c]["out"], dtype=np.float32) for c in range(8)
    ]
    out = np.stack(
        [parts[2 * b] + parts[2 * b + 1] for b in range(B)], axis=0
    )
    return out, res


def kernel(x, Wk, Wq, Wv, Wp):
    out, _ = _run(x, Wk, Wq, Wv, Wp, trace=False)
    return out


# revision 65
# speedup vs baseline: 1.1666x; 1.0106x over previous
"""Causal self-attention (B=4, T=2048, C=1024, NH=16) on 8 TRN2 NeuronCores.

Sharding: core c -> batch b = c//2, head-group g = c%2 (8 heads, Dh=512).
Each core computes q/k/v projections for its head group on its batch,
fused causal attention, and a partial output projection through its
row-slice of Wp.  Host sums the two partials per batch.

Design (all bf16 matmul inputs, fp32 psum; ~312us vs the 457us f32r
baseline; fp8 QK was tried and rejected at 3e-2 > 2e-2 tolerance):
  - Softmax exp merged per k-chunk into one [128, 1024] activation
    covering both head parities, reading a persistent double-buffered
    psum pair (qkA/qkB) so QK(kc+2) overlaps exp(kc).
  - Causal masking: tri-strip multiply on the 128-wide diagonal
    boundary only, after exp; sub-diagonal regions are simply never
    read by the (row-trimmed) AV and QK matmuls.
  - Softmax denominators ride the AV matmul via ones-columns in the v
    slots, are gathered through a staging tile + tiny SBUF-SBUF DMA
    into per-q-block [8, 512] tiles, inverted with one DVE reciprocal,
    and broadcast back to 128 rows via selection-mask matmuls.
  - Emission is software-pipelined: projection of block ib+1 and
    normalization of block jq-1 are round-robin interleaved between
    attention chunk groups; all out-projections are deferred into the
    last (ACT-heaviest) section as PE filler.
  - Per-chunk x/weight tiles + interleaved DMA order let the first
    matmul start ~1.5us in; output is written bf16 and the two
    head-group partials per batch are summed on the host in f32.

kernel(**inputs) takes the FULL unsharded inputs and returns the FULL
output.  Self-contained: hardcodes all shapes, reads nothing from disk.
"""

import sys

sys.path.insert(0, "/opt/trn_rl_repo")

import numpy as np
import ml_dtypes
from contextlib import ExitStack

import concourse.bass as bass  # noqa: F401
import concourse.mybir as mybir
import concourse.tile as tile
from concourse import bacc
from concourse.bass_utils import run_bass_kernel_spmd

P = 128
B, T, C = 4, 2048, 1024
NH, HS = 16, 64
D = 512          # per-core head dim (8 heads)
H = 8            # local heads
f32 = mybir.dt.float32
bf16 = mybir.dt.bfloat16
f8 = mybir.dt.float8e4
DR = mybir.MatmulPerfMode.DoubleRow
AFT = mybir.ActivationFunctionType


def build_nc(t=T):
    assert t % 512 == 0
    nq = t // 512     # q blocks of 512
    nkc = t // 128    # k chunks of 128
    nco = C // P      # contraction chunks (8)

    nc = bacc.Bacc("TRN2", target_bir_lowering=False, debug=False, num_devices=8)

    xt_d = nc.dram_tensor("xt", [C, t], bf16, kind="ExternalInput")
    wq_d = nc.dram_tensor("wq", [C, D], bf16, kind="ExternalInput")
    wk_d = nc.dram_tensor("wk", [C, D], bf16, kind="ExternalInput")
    wv_d = nc.dram_tensor("wv", [C, D], bf16, kind="ExternalInput")
    wp_d = nc.dram_tensor("wp", [D, C], bf16, kind="ExternalInput")
    tri_d = nc.dram_tensor("tri", [P, P], bf16, kind="ExternalInput")
    selm_d = nc.dram_tensor("selm", [8, 4, P], bf16, kind="ExternalInput")
    selm2_d = nc.dram_tensor("selm2", [2, P], bf16, kind="ExternalInput")
    out_d = nc.dram_tensor("out", [t, C], bf16, kind="ExternalOutput")

    xt_r = xt_d[:].rearrange("(co p) t -> p co t", p=P)
    wq_r = wq_d[:].rearrange("(co p) d -> p co d", p=P)
    wk_r = wk_d[:].rearrange("(co p) d -> p co d", p=P)
    wv_r = wv_d[:].rearrange("(co p) d -> p co d", p=P)
    wp_r = wp_d[:].rearrange("(dc p) c -> p dc c", p=P)
    out_r = out_d[:].rearrange("(tc p) c -> p tc c", p=P)

    with tile.TileContext(nc) as tc, ExitStack() as ctx, nc.allow_low_precision(
        reason="bf16 attention kernel"
    ):
        sb = ctx.enter_context(tc.tile_pool(name="sb", bufs=1))
        psum = ctx.enter_context(tc.tile_pool(name="psum", bufs=1, space="PSUM"))

        qt_sb = sb.tile([P, 4, t], bf16)
        kt_sb = sb.tile([P, 4, t], bf16)
        v_sb = sb.tile([P, nkc, H, P], bf16)
        yt_sb = sb.tile([P, 4, t], bf16)
        # per-jq softmax-denominator tiles: row r = hp*2 + par, par0=odd head
        sg_sb = [
            sb.tile([8, 512], f32, tag=f"sg{j}", name=f"sg{j}") for j in range(nq)
        ]
        sgi_sb = [
            sb.tile([8, 512], bf16, tag=f"sgi{j}", name=f"sgi{j}")
            for j in range(nq)
        ]
        # last q block normalizes per head-pair (hides inside attention)
        sg3 = [sb.tile([2, 512], f32, name=f"sg3_{h}") for h in range(4)]
        sgi3 = [sb.tile([2, 512], bf16, name=f"sgi3_{h}") for h in range(4)]
        tri_sb = sb.tile([P, P], bf16)
        selm_sb = sb.tile([8, 4, P], bf16)
        selm2_sb = sb.tile([2, P], bf16)
        # chunk-0 and rest-of-chunks tiles: the first matmul waits on two
        # small DMAs; everything else arrives in 3 batched DMAs (each DMA
        # issue costs ~650ns on the queue engine, so fewer is better)
        wq0_sb = sb.tile([P, D], bf16, name="wq0")
        wqr_sb = sb.tile([P, nco - 1, D], bf16, name="wqr")
        wk0_sb = sb.tile([P, D], bf16, name="wk0")
        wkr_sb = sb.tile([P, nco - 1, D], bf16, name="wkr")
        wv_sb = sb.tile([P, nco, D], bf16)
        wp_sb = sb.tile([P, 4, C], bf16)

        def wq_c(co, msl):
            return wq0_sb[:, msl] if co == 0 else wqr_sb[:, co - 1, msl]

        def wk_c(co, msl):
            return wk0_sb[:, msl] if co == 0 else wkr_sb[:, co - 1, msl]

        v5 = v_sb[:].rearrange("p k (hp par) c -> p k hp par c", par=2)

        # ---------- prelude: DMAs + memsets ----------
        # interleave x-block-0 chunks with the q/k weights they pair with
        # so the first projection matmul starts after ~2 small DMAs
        xtb0 = (
            sb.tile([P, 512], bf16, tag="xtbc0", bufs=2, name="xtb0c0"),
            sb.tile([P, nco - 1, 512], bf16, tag="xtbr", bufs=2, name="xtb0r"),
        )
        # the three DMAs the first matmuls need are issued first
        nc.sync.dma_start(xtb0[0][:], xt_r[:, 0, 0:512])
        nc.sync.dma_start(wq0_sb[:], wq_r[:, 0, :])
        nc.sync.dma_start(wk0_sb[:], wk_r[:, 0, :])
        nc.sync.dma_start(xtb0[1][:], xt_r[:, 1:nco, 0:512])
        nc.sync.dma_start(wqr_sb[:], wq_r[:, 1:nco, :])
        nc.sync.dma_start(wkr_sb[:], wk_r[:, 1:nco, :])
        nc.sync.dma_start(wv_sb[:], wv_r)
        nc.sync.dma_start(tri_sb[:], tri_d[:])
        nc.sync.dma_start(selm_sb[:], selm_d[:])
        nc.sync.dma_start(selm2_sb[:], selm2_d[:])
        nc.sync.dma_start(wp_sb[:], wp_r)
        # persistent double-buffered QK psum pair: alternating chunks use
        # A/B; stale sub-diagonal regions are re-read by exp (finite old
        # logits) and masked out after, so diag QK matmuls can trim
        qkA = psum.tile([P, 1024], f32, tag="qka", bufs=1)
        qkB = psum.tile([P, 1024], f32, tag="qkb", bufs=1)
        nc.gpsimd.memset(v_sb[:], 0.0)
        nc.gpsimd.memset(v5[:, :, :, 0, 64:65], 1.0)  # even head: s at row 64
        nc.gpsimd.memset(v5[:, :, :, 1, 0:1], 1.0)    # odd head: s at row 0
        for j in range(nq):
            nc.vector.memset(sg_sb[j][:], 1.0)
            nc.vector.memset(sgi_sb[j][:], 1.0)

        # ---------- emission units ----------
        def proj_units(ib):
            """Projection of x block ib -> qt/kt/v.  13 units."""
            if ib == 0:
                xtb = xtb0
            else:
                xtb = (
                    sb.tile([P, 512], bf16, tag="xtbc0", bufs=2,
                            name=f"xtb{ib}c0"),
                    sb.tile([P, nco - 1, 512], bf16, tag="xtbr", bufs=2,
                            name=f"xtb{ib}r"),
                )
                nc.sync.dma_start(
                    xtb[0][:], xt_r[:, 0, ib * 512 : (ib + 1) * 512]
                )
                nc.sync.dma_start(
                    xtb[1][:], xt_r[:, 1:nco, ib * 512 : (ib + 1) * 512]
                )
                yield

            def xtb_c(co, cols=slice(0, 512)):
                return (
                    xtb[0][:, cols] if co == 0 else xtb[1][:, co - 1, cols]
                )

            for m in range(4):
                for w_c, dst in ((wq_c, qt_sb), (wk_c, kt_sb)):
                    ps = psum.tile([P, 512], f32, tag="io", bufs=2)
                    for co in range(nco):
                        nc.tensor.matmul(
                            ps[:],
                            w_c(co, slice(m * P, (m + 1) * P)),
                            xtb_c(co),
                            start=(co == 0),
                            stop=(co == nco - 1),
                        )
                    nc.vector.tensor_copy(
                        out=dst[:, m, ib * 512 : (ib + 1) * 512], in_=ps[:]
                    )
                    yield
            for t4 in range(4):
                kc = ib * 4 + t4
                ps = psum.tile([P, 512], f32, tag="io", bufs=2)
                for co in range(nco):
                    nc.tensor.matmul(
                        ps[:],
                        xtb_c(co, slice(t4 * P, (t4 + 1) * P)),
                        wv_sb[:, co, :],
                        start=(co == 0),
                        stop=(co == nco - 1),
                    )
                src = ps[:].rearrange("p (hp par c) -> p hp par c", par=2, c=64)
                nc.vector.tensor_copy(out=v5[:, kc, :, 0, 0:64], in_=src[:, :, 0, :])
                nc.vector.tensor_copy(
                    out=v5[:, kc, :, 1, 64:128], in_=src[:, :, 1, :]
                )
                yield

        def att_units(hp, jq):
            """Attention for head-pair hp on q block jq.

            One k-chunk per group: QK-E + QK-O into a double-buffered
            [P, 1024] psum, one exp over both parities.  Diagonal chunks
            get small memset + tri-strip masking (off the QK->exp chain).
            AV trails the exp by one chunk.
            """
            q0 = jq * 512
            nk = 4 * (jq + 1)
            psyE = psum.tile([P, 512], f32, tag="ye", bufs=1)
            psyO = psum.tile([P, 512], f32, tag="yo", bufs=1)

            def av_emit(kc, attg):
                d = kc - 4 * jq
                off = 128 * d if d >= 0 else 0
                for par, psy in ((0, psyE), (1, psyO)):
                    nc.tensor.matmul(
                        psy[:, off:512],
                        v_sb[:, kc, 2 * hp + par, :],
                        attg[:, par * 512 + off : par * 512 + 512],
                        start=(kc == 0),
                        stop=(kc == nk - 1),
                        skip_group_check=True,
                    )

            prev = None
            for kc in range(nk):
                d = kc - 4 * jq
                # trim diag QK on jq>0 (jq=0 writes full so the psum pair's
                # first generations are fully initialized)
                toff = 128 * d if (d > 0 and jq > 0) else 0
                qk = qkA if kc % 2 == 0 else qkB
                for par, sl in ((0, slice(0, 64)), (1, slice(64, 128))):
                    col = par * 512
                    nc.tensor.matmul(
                        qk[:, col + toff : col + 512],
                        kt_sb[sl, hp, kc * P : (kc + 1) * P],
                        qt_sb[sl, hp, q0 + toff : q0 + 512],
                        start=True,
                        stop=True,
                    )
                attg = sb.tile([P, 1024], bf16, tag="att", bufs=4)
                nc.scalar.activation(attg[:], qk[:], AFT.Exp, scale=0.125)
                if d >= 0:  # diagonal chunk -> causal tri mask on the
                    # boundary strip; cols below off are never read by AV
                    off = 128 * d
                    for par in (0, 1):
                        col = par * 512
                        eng = nc.vector if par == 0 else nc.gpsimd
                        eng.tensor_mul(
                            out=attg[:, col + off : col + off + P],
                            in0=attg[:, col + off : col + off + P],
                            in1=tri_sb[:],
                        )
                if prev is not None:
                    av_emit(*prev)
                prev = (kc, attg)
                yield
            av_emit(*prev)
            # drains: yt + s rows (staged, then a tiny DMA gathers psum
            # rows {0 (odd s), 64 (even s)} into adjacent sg rows)
            last = jq == nq - 1
            nc.vector.tensor_copy(
                out=yt_sb[0:64, hp, q0 : q0 + 512], in_=psyE[0:64, :]
            )
            stmp = sb.tile([P, 512], f32, tag="stmp", bufs=2)
            if last and hp == 3:
                # tail-critical: split drains across DVE and the (by now
                # idle) scalar engine
                nc.scalar.copy(
                    out=yt_sb[64:128, hp, q0 : q0 + 512], in_=psyO[64:128, :]
                )
                nc.scalar.copy(out=stmp[64:65, :], in_=psyE[64:65, :])
            else:
                nc.vector.tensor_copy(
                    out=yt_sb[64:128, hp, q0 : q0 + 512], in_=psyO[64:128, :]
                )
                nc.vector.tensor_copy(out=stmp[64:65, :], in_=psyE[64:65, :])
            nc.vector.tensor_copy(out=stmp[0:1, :], in_=psyO[0:1, :])
            if last:
                # normalize this head pair now; it overlaps the remaining
                # head pairs' attention instead of serializing at the end
                nc.sync.dma_start(sg3[hp][0:2, :], stmp[0:65:64, :])
                nc.vector.reciprocal(out=sgi3[hp][:], in_=sg3[hp][:])
                rb = psum.tile([P, 512], f32, tag="io", bufs=2)
                nc.tensor.matmul(
                    rb[:], selm2_sb[:], sgi3[hp][:], start=True, stop=True
                )
                nc.vector.tensor_mul(
                    out=yt_sb[:, hp, q0 : q0 + 512],
                    in0=yt_sb[:, hp, q0 : q0 + 512],
                    in1=rb[:],
                )
            else:
                nc.sync.dma_start(
                    sg_sb[jq][hp * 2 : hp * 2 + 2, :], stmp[0:65:64, :]
                )
            yield

        def tail_units(jq):
            yield from norm_units(jq)
            yield from outp_units(jq)

        def norm_units(jq):
            """Normalization (1/s broadcast + scale) for q block jq."""
            q0 = jq * 512
            # 1/s on DVE (keeps the ACT table set pinned to exp)
            nc.vector.reciprocal(out=sgi_sb[jq][:], in_=sg_sb[jq][:])
            yield
            for hp in range(4):
                rb = psum.tile([P, 512], f32, tag="io", bufs=2)
                nc.tensor.matmul(
                    rb[:],
                    selm_sb[:, hp, :],
                    sgi_sb[jq][:],
                    start=True,
                    stop=True,
                )
                nc.vector.tensor_mul(
                    out=yt_sb[:, hp, q0 : q0 + 512],
                    in0=yt_sb[:, hp, q0 : q0 + 512],
                    in1=rb[:],
                )
                yield

        def outp_units(jq):
            """Out-projection for q block jq (deferrable filler work)."""
            for tcn in range(jq * 4, jq * 4 + 4):
                ob = sb.tile([P, C], bf16, tag="ob", bufs=3)
                for n2 in (0, 1):
                    pso = psum.tile([P, 512], f32, tag="io", bufs=2)
                    for dc in range(4):
                        nc.tensor.matmul(
                            pso[:],
                            yt_sb[:, dc, tcn * P : (tcn + 1) * P],
                            wp_sb[:, dc, n2 * 512 : (n2 + 1) * 512],
                            start=(dc == 0),
                            stop=(dc == 3),
                        )
                    if n2 == 0:
                        nc.vector.tensor_copy(out=ob[:, 0:512], in_=pso[:])
                    else:
                        nc.scalar.copy(out=ob[:, 512:1024], in_=pso[:])
                eng = nc.sync if tcn % 2 == 0 else nc.scalar
                eng.dma_start(out_r[:, tcn, :], ob[:])
                yield

        def chain(*gens):
            for g in gens:
                yield from g

        def roundrobin(*gens):
            gens = list(gens)
            while gens:
                for g in list(gens):
                    try:
                        next(g)
                        yield
                    except StopIteration:
                        gens.remove(g)

        def run_merged(main_gens, filler_gens, n_main, n_fill):
            main = chain(*main_gens)
            fill = chain(*filler_gens)
            ratio = (n_fill / n_main) if n_main else 0.0
            acc = 0.0
            done = object()
            fill_done = n_fill == 0
            for _ in main:
                acc += ratio
                while acc >= 1.0 and not fill_done:
                    fill_done = next(fill, done) is done
                    acc -= 1.0
            while not fill_done:
                fill_done = next(fill, done) is done

        # ---------- emission schedule ----------
        for _ in proj_units(0):
            pass
        for jq in range(nq):
            mains = [att_units(hp, jq) for hp in range(4)]
            n_main = 4 * (4 * (jq + 1) + 1)
            fillers = []
            n_fill = 0
            if jq + 1 < nq:
                fillers.append(proj_units(jq + 1))
                n_fill += 13
            if jq >= 1:
                fillers.append(norm_units(jq - 1))
                n_fill += 5
            if jq == nq - 1:
                # out-projections deferred here: the last section has the
                # most ACT (exp) work and the least other PE filler
                for j2 in range(nq - 1):
                    fillers.append(outp_units(j2))
                    n_fill += 4
            if len(fillers) > 1:
                fillers = [chain(*fillers)] if jq == nq - 1 else [
                    roundrobin(*fillers)
                ]
            run_merged(mains, fillers, n_main, n_fill)
        for _ in outp_units(nq - 1):
            pass

    nc.finalize()
    return nc


_NC = None


def _get_nc():
    global _NC
    if _NC is None:
        _NC = build_nc()
    return _NC


def make_in_maps(x, Wk, Wq, Wv, Wp, t=T):
    x = np.asarray(x, dtype=np.float32)
    Wk = np.asarray(Wk, dtype=np.float32)
    Wq = np.asarray(Wq, dtype=np.float32)
    Wv = np.asarray(Wv, dtype=np.float32)
    Wp = np.asarray(Wp, dtype=np.float32)
    bf = ml_dtypes.bfloat16

    # lower-tri strip mask: tri[k, j] = 1 iff j >= k
    tri = np.triu(np.ones((P, P), np.float32)).astype(bf)

    # selection masks for broadcasting 1/s rows; sg row hp*2+0 holds the
    # ODD head's s (psum row 0), hp*2+1 the even head's (psum row 64)
    selm = np.zeros((8, 4, P), np.float32)
    for hp in range(4):
        selm[hp * 2 + 0, hp, 64:128] = 1.0
        selm[hp * 2 + 1, hp, 0:64] = 1.0
    selm = selm.astype(bf)
    selm2 = np.zeros((2, P), np.float32)
    selm2[0, 64:128] = 1.0  # row 0 = odd head's s
    selm2[1, 0:64] = 1.0    # row 1 = even head's s
    selm2 = selm2.astype(bf)

    in_maps = []
    for c in range(8):
        b, g = c // 2, c % 2
        sl = slice(g * D, (g + 1) * D)
        in_maps.append({
            "xt": np.ascontiguousarray(x[b, :t].T).astype(bf),
            "wq": np.ascontiguousarray(Wq[:, sl]).astype(bf),
            "wk": np.ascontiguousarray(Wk[:, sl]).astype(bf),
            "wv": np.ascontiguousarray(Wv[:, sl]).astype(bf),
            "wp": np.ascontiguousarray(Wp[sl, :]).astype(bf),
            "tri": tri,
            "selm": selm,
            "selm2": selm2,
        })
    return in_maps


def _run(x, Wk, Wq, Wv, Wp, trace=False):
    nc = _get_nc()
    in_maps = make_in_maps(x, Wk, Wq, Wv, Wp)
    res = run_bass_kernel_spmd(nc, in_maps, core_ids=list(range(8)), trace=trace)
    parts = [
        np.asarray(res.results[c]["out"], dtype=np.float32) for c in range(8)
    ]
    out = np.stack(
        [parts[2 * b] + parts[2 * b + 1] for b in range(B)], axis=0
    )
    return out, res


def kernel(x, Wk, Wq, Wv, Wp):
    out, _ = _run(x, Wk, Wq, Wv, Wp, trace=False)
    return out


# revision 68
# speedup vs baseline: 1.1801x; 1.0115x over previous
"""Causal self-attention (B=4, T=2048, C=1024, NH=16) on 8 TRN2 NeuronCores.

Sharding: core c -> batch b = c//2, head-group g = c%2 (8 heads, Dh=512).
Each core computes q/k/v projections for its head group on its batch,
fused causal attention, and a partial output projection through its
row-slice of Wp.  Host sums the two partials per batch.

Design (all bf16 matmul inputs, fp32 psum; ~312us vs the 457us f32r
baseline; fp8 QK was tried and rejected at 3e-2 > 2e-2 tolerance):
  - Softmax exp merged per k-chunk into one [128, 1024] activation
    covering both head parities, reading a persistent double-buffered
    psum pair (qkA/qkB) so QK(kc+2) overlaps exp(kc).
  - Causal masking: tri-strip multiply on the 128-wide diagonal
    boundary only, after exp; sub-diagonal regions are simply never
    read by the (row-trimmed) AV and QK matmuls.
  - Softmax denominators ride the AV matmul via ones-columns in the v
    slots, are gathered through a staging tile + tiny SBUF-SBUF DMA
    into per-q-block [8, 512] tiles, inverted with one DVE reciprocal,
    and broadcast back to 128 rows via selection-mask matmuls.
  - Emission is software-pipelined: projection of block ib+1 and
    normalization of block jq-1 are round-robin interleaved between
    attention chunk groups; all out-projections are deferred into the
    last (ACT-heaviest) section as PE filler.
  - First-needed x/weight chunks get their own tiles and DMAs (issue
    order first; each DMA issue costs ~650ns on the queue engine, so
    the remaining chunks ride 3 batched DMAs); the last q block
    normalizes per head-pair so the softmax-denominator chain hides
    inside attention instead of serializing at the tail; output is
    written bf16 and the two head-group partials per batch are summed
    on the host in f32.

kernel(**inputs) takes the FULL unsharded inputs and returns the FULL
output.  Self-contained: hardcodes all shapes, reads nothing from disk.
"""

import sys

sys.path.insert(0, "/opt/trn_rl_repo")

import numpy as np
import ml_dtypes
from contextlib import ExitStack

import concourse.bass as bass  # noqa: F401
import concourse.mybir as mybir
import concourse.tile as tile
from concourse import bacc
from concourse.bass_utils import run_bass_kernel_spmd

P = 128
B, T, C = 4, 2048, 1024
NH, HS = 16, 64
D = 512          # per-core head dim (8 heads)
H = 8            # local heads
f32 = mybir.dt.float32
bf16 = mybir.dt.bfloat16
f8 = mybir.dt.float8e4
DR = mybir.MatmulPerfMode.DoubleRow
AFT = mybir.ActivationFunctionType


def build_nc(t=T):
    assert t % 512 == 0
    nq = t // 512     # q blocks of 512
    nkc = t // 128    # k chunks of 128
    nco = C // P      # contraction chunks (8)

    nc = bacc.Bacc("TRN2", target_bir_lowering=False, debug=False, num_devices=8)

    xt_d = nc.dram_tensor("xt", [C, t], bf16, kind="ExternalInput")
    wq_d = nc.dram_tensor("wq", [C, D], bf16, kind="ExternalInput")
    wk_d = nc.dram_tensor("wk", [C, D], bf16, kind="ExternalInput")
    wv_d = nc.dram_tensor("wv", [C, D], bf16, kind="ExternalInput")
    wp_d = nc.dram_tensor("wp", [D, C], bf16, kind="ExternalInput")
    tri_d = nc.dram_tensor("tri", [P, P], bf16, kind="ExternalInput")
    selm_d = nc.dram_tensor("selm", [8, 4, P], bf16, kind="ExternalInput")
    selm2_d = nc.dram_tensor("selm2", [2, P], bf16, kind="ExternalInput")
    out_d = nc.dram_tensor("out", [t, C], bf16, kind="ExternalOutput")

    xt_r = xt_d[:].rearrange("(co p) t -> p co t", p=P)
    wq_r = wq_d[:].rearrange("(co p) d -> p co d", p=P)
    wk_r = wk_d[:].rearrange("(co p) d -> p co d", p=P)
    wv_r = wv_d[:].rearrange("(co p) d -> p co d", p=P)
    wp_r = wp_d[:].rearrange("(dc p) c -> p dc c", p=P)
    out_r = out_d[:].rearrange("(tc p) c -> p tc c", p=P)

    with tile.TileContext(nc) as tc, ExitStack() as ctx, nc.allow_low_precision(
        reason="bf16 attention kernel"
    ):
        sb = ctx.enter_context(tc.tile_pool(name="sb", bufs=1))
        psum = ctx.enter_context(tc.tile_pool(name="psum", bufs=1, space="PSUM"))

        qt_sb = sb.tile([P, 4, t], bf16)
        kt_sb = sb.tile([P, 4, t], bf16)
        v_sb = sb.tile([P, nkc, H, P], bf16)
        yt_sb = sb.tile([P, 4, t], bf16)
        # per-jq softmax-denominator tiles: row r = hp*2 + par, par0=odd head
        sg_sb = [
            sb.tile([8, 512], f32, tag=f"sg{j}", name=f"sg{j}") for j in range(nq)
        ]
        sgi_sb = [
            sb.tile([8, 512], bf16, tag=f"sgi{j}", name=f"sgi{j}")
            for j in range(nq)
        ]
        # last q block normalizes per head-pair (hides inside attention)
        sg3 = [sb.tile([2, 512], f32, name=f"sg3_{h}") for h in range(4)]
        sgi3 = [sb.tile([2, 512], bf16, name=f"sgi3_{h}") for h in range(4)]
        tri_sb = sb.tile([P, P], bf16)
        selm_sb = sb.tile([8, 4, P], bf16)
        selm2_sb = sb.tile([2, P], bf16)
        # chunk-0 and rest-of-chunks tiles: the first matmul waits on two
        # small DMAs; everything else arrives in 3 batched DMAs (each DMA
        # issue costs ~650ns on the queue engine, so fewer is better)
        wq0_sb = sb.tile([P, D], bf16, name="wq0")
        wqr_sb = sb.tile([P, nco - 1, D], bf16, name="wqr")
        wk0_sb = sb.tile([P, D], bf16, name="wk0")
        wkr_sb = sb.tile([P, nco - 1, D], bf16, name="wkr")
        wv_sb = sb.tile([P, nco, D], bf16)
        wp_sb = sb.tile([P, 4, C], bf16)

        def wq_c(co, msl):
            return wq0_sb[:, msl] if co == 0 else wqr_sb[:, co - 1, msl]

        def wk_c(co, msl):
            return wk0_sb[:, msl] if co == 0 else wkr_sb[:, co - 1, msl]

        v5 = v_sb[:].rearrange("p k (hp par) c -> p k hp par c", par=2)

        # ---------- prelude: DMAs + memsets ----------
        # interleave x-block-0 chunks with the q/k weights they pair with
        # so the first projection matmul starts after ~2 small DMAs
        xtb0 = (
            sb.tile([P, 512], bf16, tag="xtbc0", bufs=2, name="xtb0c0"),
            sb.tile([P, nco - 1, 512], bf16, tag="xtbr", bufs=2, name="xtb0r"),
        )
        # the three DMAs the first matmuls need are issued first
        nc.sync.dma_start(xtb0[0][:], xt_r[:, 0, 0:512])
        nc.sync.dma_start(wq0_sb[:], wq_r[:, 0, :])
        nc.sync.dma_start(wk0_sb[:], wk_r[:, 0, :])
        nc.sync.dma_start(xtb0[1][:], xt_r[:, 1:nco, 0:512])
        nc.sync.dma_start(wqr_sb[:], wq_r[:, 1:nco, :])
        nc.sync.dma_start(wkr_sb[:], wk_r[:, 1:nco, :])
        nc.sync.dma_start(wv_sb[:], wv_r)
        nc.sync.dma_start(tri_sb[:], tri_d[:])
        nc.sync.dma_start(selm_sb[:], selm_d[:])
        nc.sync.dma_start(selm2_sb[:], selm2_d[:])
        nc.sync.dma_start(wp_sb[:], wp_r)
        # persistent double-buffered QK psum pair: alternating chunks use
        # A/B; stale sub-diagonal regions are re-read by exp (finite old
        # logits) and masked out after, so diag QK matmuls can trim
        qkA = psum.tile([P, 1024], f32, tag="qka", bufs=1)
        qkB = psum.tile([P, 1024], f32, tag="qkb", bufs=1)
        nc.gpsimd.memset(v_sb[:], 0.0)
        nc.gpsimd.memset(v5[:, :, :, 0, 64:65], 1.0)  # even head: s at row 64
        nc.gpsimd.memset(v5[:, :, :, 1, 0:1], 1.0)    # odd head: s at row 0
        for j in range(nq):
            nc.vector.memset(sg_sb[j][:], 1.0)
            nc.vector.memset(sgi_sb[j][:], 1.0)

        # ---------- emission units ----------
        def proj_units(ib):
            """Projection of x block ib -> qt/kt/v.  13 units."""
            if ib == 0:
                xtb = xtb0
            else:
                xtb = (
                    sb.tile([P, 512], bf16, tag="xtbc0", bufs=2,
                            name=f"xtb{ib}c0"),
                    sb.tile([P, nco - 1, 512], bf16, tag="xtbr", bufs=2,
                            name=f"xtb{ib}r"),
                )
                nc.sync.dma_start(
                    xtb[0][:], xt_r[:, 0, ib * 512 : (ib + 1) * 512]
                )
                nc.sync.dma_start(
                    xtb[1][:], xt_r[:, 1:nco, ib * 512 : (ib + 1) * 512]
                )
                yield

            def xtb_c(co, cols=slice(0, 512)):
                return (
                    xtb[0][:, cols] if co == 0 else xtb[1][:, co - 1, cols]
                )

            for m in range(4):
                for w_c, dst in ((wq_c, qt_sb), (wk_c, kt_sb)):
                    ps = psum.tile([P, 512], f32, tag="io", bufs=2)
                    for co in range(nco):
                        nc.tensor.matmul(
                            ps[:],
                            w_c(co, slice(m * P, (m + 1) * P)),
                            xtb_c(co),
                            start=(co == 0),
                            stop=(co == nco - 1),
                        )
                    nc.vector.tensor_copy(
                        out=dst[:, m, ib * 512 : (ib + 1) * 512], in_=ps[:]
                    )
                    yield
            for t4 in range(4):
                kc = ib * 4 + t4
                ps = psum.tile([P, 512], f32, tag="io", bufs=2)
                for co in range(nco):
                    nc.tensor.matmul(
                        ps[:],
                        xtb_c(co, slice(t4 * P, (t4 + 1) * P)),
                        wv_sb[:, co, :],
                        start=(co == 0),
                        stop=(co == nco - 1),
                    )
                src = ps[:].rearrange("p (hp par c) -> p hp par c", par=2, c=64)
                nc.vector.tensor_copy(out=v5[:, kc, :, 0, 0:64], in_=src[:, :, 0, :])
                nc.vector.tensor_copy(
                    out=v5[:, kc, :, 1, 64:128], in_=src[:, :, 1, :]
                )
                yield

        def att_units(hp, jq):
            """Attention for head-pair hp on q block jq.

            One k-chunk per group: QK-E + QK-O into a double-buffered
            [P, 1024] psum, one exp over both parities.  Diagonal chunks
            get small memset + tri-strip masking (off the QK->exp chain).
            AV trails the exp by one chunk.
            """
            q0 = jq * 512
            nk = 4 * (jq + 1)
            psyE = psum.tile([P, 512], f32, tag="ye", bufs=1)
            psyO = psum.tile([P, 512], f32, tag="yo", bufs=1)

            def av_emit(kc, attg):
                d = kc - 4 * jq
                off = 128 * d if d >= 0 else 0
                for par, psy in ((0, psyE), (1, psyO)):
                    nc.tensor.matmul(
                        psy[:, off:512],
                        v_sb[:, kc, 2 * hp + par, :],
                        attg[:, par * 512 + off : par * 512 + 512],
                        start=(kc == 0),
                        stop=(kc == nk - 1),
                        skip_group_check=True,
                    )

            prev = None
            for kc in range(nk):
                d = kc - 4 * jq
                # trim diag QK on jq>0 (jq=0 writes full so the psum pair's
                # first generations are fully initialized)
                toff = 128 * d if (d > 0 and jq > 0) else 0
                qk = qkA if kc % 2 == 0 else qkB
                for par, sl in ((0, slice(0, 64)), (1, slice(64, 128))):
                    col = par * 512
                    nc.tensor.matmul(
                        qk[:, col + toff : col + 512],
                        kt_sb[sl, hp, kc * P : (kc + 1) * P],
                        qt_sb[sl, hp, q0 + toff : q0 + 512],
                        start=True,
                        stop=True,
                    )
                attg = sb.tile([P, 1024], bf16, tag="att", bufs=4)
                nc.scalar.activation(attg[:], qk[:], AFT.Exp, scale=0.125)
                if d >= 0:  # diagonal chunk -> causal tri mask on the
                    # boundary strip; cols below off are never read by AV
                    off = 128 * d
                    for par in (0, 1):
                        col = par * 512
                        eng = nc.vector if par == 0 else nc.gpsimd
                        eng.tensor_mul(
                            out=attg[:, col + off : col + off + P],
                            in0=attg[:, col + off : col + off + P],
                            in1=tri_sb[:],
                        )
                if prev is not None:
                    av_emit(*prev)
                prev = (kc, attg)
                yield
            av_emit(*prev)
            # drains: yt + s rows (staged, then a tiny DMA gathers psum
            # rows {0 (odd s), 64 (even s)} into adjacent sg rows)
            last = jq == nq - 1
            nc.vector.tensor_copy(
                out=yt_sb[0:64, hp, q0 : q0 + 512], in_=psyE[0:64, :]
            )
            stmp = sb.tile([P, 512], f32, tag="stmp", bufs=2)
            if last and hp == 3:
                # tail-critical: split drains across DVE and the (by now
                # idle) scalar engine
                nc.scalar.copy(
                    out=yt_sb[64:128, hp, q0 : q0 + 512], in_=psyO[64:128, :]
                )
                nc.scalar.copy(out=stmp[64:65, :], in_=psyE[64:65, :])
            else:
                nc.vector.tensor_copy(
                    out=yt_sb[64:128, hp, q0 : q0 + 512], in_=psyO[64:128, :]
                )
                nc.vector.tensor_copy(out=stmp[64:65, :], in_=psyE[64:65, :])
            nc.vector.tensor_copy(out=stmp[0:1, :], in_=psyO[0:1, :])
            if last:
                # normalize this head pair now; it overlaps the remaining
                # head pairs' attention instead of serializing at the end
                nc.sync.dma_start(sg3[hp][0:2, :], stmp[0:65:64, :])
                nc.vector.reciprocal(out=sgi3[hp][:], in_=sg3[hp][:])
                rb = psum.tile([P, 512], f32, tag="io", bufs=2)
                nc.tensor.matmul(
                    rb[:], selm2_sb[:], sgi3[hp][:], start=True, stop=True
                )
                nc.vector.tensor_mul(
                    out=yt_sb[:, hp, q0 : q0 + 512],
                    in0=yt_sb[:, hp, q0 : q0 + 512],
                    in1=rb[:],
                )
            else:
                nc.sync.dma_start(
                    sg_sb[jq][hp * 2 : hp * 2 + 2, :], stmp[0:65:64, :]
                )
            yield

        def tail_units(jq):
            yield from norm_units(jq)
            yield from outp_units(jq)

        def norm_units(jq):
            """Normalization (1/s broadcast + scale) for q block jq."""
            q0 = jq * 512
            # 1/s on DVE (keeps the ACT table set pinned to exp)
            nc.vector.reciprocal(out=sgi_sb[jq][:], in_=sg_sb[jq][:])
            yield
            for hp in range(4):
                rb = psum.tile([P, 512], f32, tag="io", bufs=2)
                nc.tensor.matmul(
                    rb[:],
                    selm_sb[:, hp, :],
                    sgi_sb[jq][:],
                    start=True,
                    stop=True,
                )
                nc.vector.tensor_mul(
                    out=yt_sb[:, hp, q0 : q0 + 512],
                    in0=yt_sb[:, hp, q0 : q0 + 512],
                    in1=rb[:],
                )
                yield

        def outp_units(jq):
            """Out-projection for q block jq (deferrable filler work)."""
            if jq == nq - 1:
                yield from outp_tail(jq)
                return
            for tcn in range(jq * 4, jq * 4 + 4):
                ob = sb.tile([P, C], bf16, tag="ob", bufs=3)
                for n2 in (0, 1):
                    pso = psum.tile([P, 512], f32, tag="io", bufs=2)
                    for dc in range(4):
                        nc.tensor.matmul(
                            pso[:],
                            yt_sb[:, dc, tcn * P : (tcn + 1) * P],
                            wp_sb[:, dc, n2 * 512 : (n2 + 1) * 512],
                            start=(dc == 0),
                            stop=(dc == 3),
                        )
                    if n2 == 0:
                        nc.vector.tensor_copy(out=ob[:, 0:512], in_=pso[:])
                    else:
                        nc.scalar.copy(out=ob[:, 512:1024], in_=pso[:])
                eng = nc.sync if tcn % 2 == 0 else nc.scalar
                eng.dma_start(out_r[:, tcn, :], ob[:])
                yield

        def outp_tail(jq):
            """Final-block out-projection: attention is done, so all 8
            psum banks are free -> open all 8 accumulation groups at
            once.  The dc<3 matmuls pre-run during the last head pair's
            normalization; only 8 dc=3 matmuls trail it."""
            slots = [
                psum.tile([P, 512], f32, tag="io", bufs=2, name="ot0")[:],
                psum.tile([P, 512], f32, tag="io", bufs=2, name="ot1")[:],
                qkA[:, 0:512],
                qkA[:, 512:1024],
                qkB[:, 0:512],
                qkB[:, 512:1024],
                psum.tile([P, 512], f32, tag="ye", bufs=1, name="ot6")[:],
                psum.tile([P, 512], f32, tag="yo", bufs=1, name="ot7")[:],
            ]
            pairs = [
                (tcn, n2)
                for tcn in range(jq * 4, jq * 4 + 4)
                for n2 in (0, 1)
            ]
            for dc in range(3):
                for g, (tcn, n2) in enumerate(pairs):
                    nc.tensor.matmul(
                        slots[g],
                        yt_sb[:, dc, tcn * P : (tcn + 1) * P],
                        wp_sb[:, dc, n2 * 512 : (n2 + 1) * 512],
                        start=(dc == 0),
                        stop=False,
                        skip_group_check=True,
                    )
                yield
            obs = {}
            for tcn in range(jq * 4, jq * 4 + 4):
                obs[tcn] = sb.tile(
                    [P, C], bf16, tag="obt", bufs=4, name=f"obt{tcn}"
                )
            for g, (tcn, n2) in enumerate(pairs):
                nc.tensor.matmul(
                    slots[g],
                    yt_sb[:, 3, tcn * P : (tcn + 1) * P],
                    wp_sb[:, 3, n2 * 512 : (n2 + 1) * 512],
                    start=False,
                    stop=True,
                    skip_group_check=True,
                )
                if n2 == 0:
                    nc.vector.tensor_copy(
                        out=obs[tcn][:, 0:512], in_=slots[g]
                    )
                else:
                    nc.scalar.copy(out=obs[tcn][:, 512:1024], in_=slots[g])
                    de = nc.sync if tcn % 2 == 0 else nc.scalar
                    de.dma_start(out_r[:, tcn, :], obs[tcn][:])
                yield

        def chain(*gens):
            for g in gens:
                yield from g

        def roundrobin(*gens):
            gens = list(gens)
            while gens:
                for g in list(gens):
                    try:
                        next(g)
                        yield
                    except StopIteration:
                        gens.remove(g)

        def run_merged(main_gens, filler_gens, n_main, n_fill):
            main = chain(*main_gens)
            fill = chain(*filler_gens)
            ratio = (n_fill / n_main) if n_main else 0.0
            acc = 0.0
            done = object()
            fill_done = n_fill == 0
            for _ in main:
                acc += ratio
                while acc >= 1.0 and not fill_done:
                    fill_done = next(fill, done) is done
                    acc -= 1.0
            while not fill_done:
                fill_done = next(fill, done) is done

        # ---------- emission schedule ----------
        for _ in proj_units(0):
            pass
        for jq in range(nq):
            mains = [att_units(hp, jq) for hp in range(4)]
            n_main = 4 * (4 * (jq + 1) + 1)
            fillers = []
            n_fill = 0
            if jq + 1 < nq:
                fillers.append(proj_units(jq + 1))
                n_fill += 13
            if jq >= 1:
                fillers.append(norm_units(jq - 1))
                n_fill += 5
            if jq == nq - 1:
                # out-projections deferred here: the last section has the
                # most ACT (exp) work and the least other PE filler
                for j2 in range(nq - 1):
                    fillers.append(outp_units(j2))
                    n_fill += 4
            if len(fillers) > 1:
                fillers = [chain(*fillers)] if jq == nq - 1 else [
                    roundrobin(*fillers)
                ]
            run_merged(mains, fillers, n_main, n_fill)
        for _ in outp_units(nq - 1):
            pass

    nc.finalize()
    return nc


_NC = None


def _get_nc():
    global _NC
    if _NC is None:
        _NC = build_nc()
    return _NC


def make_in_maps(x, Wk, Wq, Wv, Wp, t=T):
    x = np.asarray(x, dtype=np.float32)
    Wk = np.asarray(Wk, dtype=np.float32)
    Wq = np.asarray(Wq, dtype=np.float32)
    Wv = np.asarray(Wv, dtype=np.float32)
    Wp = np.asarray(Wp, dtype=np.float32)
    bf = ml_dtypes.bfloat16

    # lower-tri strip mask: tri[k, j] = 1 iff j >= k
    tri = np.triu(np.ones((P, P), np.float32)).astype(bf)

    # selection masks for broadcasting 1/s rows; sg row hp*2+0 holds the
    # ODD head's s (psum row 0), hp*2+1 the even head's (psum row 64)
    selm = np.zeros((8, 4, P), np.float32)
    for hp in range(4):
        selm[hp * 2 + 0, hp, 64:128] = 1.0
        selm[hp * 2 + 1, hp, 0:64] = 1.0
    selm = selm.astype(bf)
    selm2 = np.zeros((2, P), np.float32)
    selm2[0, 64:128] = 1.0  # row 0 = odd head's s
    selm2[1, 0:64] = 1.0    # row 1 = even head's s
    selm2 = selm2.astype(bf)

    in_maps = []
    for c in range(8):
        b, g = c // 2, c % 2
        sl = slice(g * D, (g + 1) * D)
        in_maps.append({
            "xt": np.ascontiguousarray(x[b, :t].T).astype(bf),
            "wq": np.ascontiguousarray(Wq[:, sl]).astype(bf),
            "wk": np.ascontiguousarray(Wk[:, sl]).astype(bf),
            "wv": np.ascontiguousarray(Wv[:, sl]).astype(bf),
            "wp": np.ascontiguousarray(Wp[sl, :]).astype(bf),
            "tri": tri,
            "selm": selm,
            "selm2": selm2,
        })
    return in_maps


def _run(x, Wk, Wq, Wv, Wp, trace=False):
    nc = _get_nc()
    in_maps = make_in_maps(x, Wk, Wq, Wv, Wp)
    res = run_bass_kernel_spmd(nc, in_maps, core_ids=list(range(8)), trace=trace)
    parts = [
        np.asarray(res.results[c]["out"], dtype=np.float32) for c in range(8)
    ]
    out = np.stack(
        [parts[2 * b] + parts[2 * b + 1] for b in range(B)], axis=0
    )
    return out, res


def kernel(x, Wk, Wq, Wv, Wp):
    out, _ = _run(x, Wk, Wq, Wv, Wp, trace=False)
    return out
